# revision 4
# baseline (speedup 1.0000x reference)
"""GBST Trainium2 kernel v2 (nn_GBST_42434276884940).

Self-contained: takes FULL inputs, shards batch over 8 NeuronCores
(2 rows/core), runs a Bass/Tile kernel per core, gathers full output.

v2 redesign vs v1: the indirect-DMA gather (measured ~205ns/row on HW,
~1.26ms total — the v1 bottleneck) is replaced by an on-device one-hot
matmul gather: oh[v,q] = (id[q]==v) built on DVE (int16 compare, 2-byte
4x mode), then X[pos,:] = oh.T @ taug accumulated over the two 128-row
vocab halves on the PE in bf16 (1 cyc/row vs fp32's 4). Positional
encoding is per-partition-constant (phase = p%4) and folded in either
via a K=4 accumulating matmul (pe4/ph4t) or a fused PSUM-evict add of
peadd, chosen per chunk to balance PE/DVE/ACT/Pool load. All heavy
matmuls (G build + banded apply) and elementwise run in bf16
(gate is 2e-2; bf16 path measures ~1e-3).
"""

import sys

import numpy as np
import ml_dtypes

if "/opt/trn_rl_repo" not in sys.path:
    sys.path.insert(0, "/opt/trn_rl_repo")

import concourse.bass as bass
import concourse.tile as tile
from concourse import bacc, mybir
from concourse.bass_utils import run_bass_kernel_spmd

F32 = mybir.dt.float32
BF16 = mybir.dt.bfloat16
I16 = mybir.dt.int16
BF = ml_dtypes.bfloat16

MAX_BLOCK = 4
EMBED = 256
VOCAB = 256
BATCH = 16
SEQ = 3072
NCORES = 8
BLOC = BATCH // NCORES           # 2
NPOS = BLOC * SEQ                # 6144
NCHUNK = NPOS // 128             # 48
NGROUP = 4
GSZ = NCHUNK // NGROUP           # 12
NELEM = 257                      # 256 embed + 1 score col

SLOTS = [0, 3, 6, 9, 2, 5, 8, 11, 1, 4, 7, 10]   # slot s -> tau_l
SLOT_OF = {t: s for s, t in enumerate(SLOTS)}
CLASS_TAUL = [[0, 3, 6, 9], [2, 5, 8, 11], [1, 4, 7, 10]]

# per-chunk evict mode: 0 = DVE tensor_tensor add of fp32 peadd,
# 1 = phase-matmul + ACT copy, 2 = phase-matmul + Pool copy
EVICT_MODE = [0 if (c % 7) in (0, 3) else 1 for c in range(NCHUNK)]
# per-chunk one-hot compare engine: 0=DVE 1=Pool
CMP_ENG = [0 for c in range(NCHUNK)]  # TSP compares, DVE only
# per-ot engine for out2->osb copy: 0=ACT 1=DVE 2=Pool
OSB_ENG = [1 if pt < 4 else 0 for pt in range(7)]  # per ot-PAIR: 0=ACT 1=DVE


# ---------------------------------------------------------------- host consts

def _sinusoidal_pe(max_len, d):
    pos = np.arange(max_len, dtype=np.float32)[:, None]
    div = np.exp(np.arange(0, d, 2, dtype=np.float32) * (-np.log(10000.0) / d))
    pe = np.zeros((max_len, d), dtype=np.float32)
    pe[:, 0::2] = np.sin(pos * div)
    pe[:, 1::2] = np.cos(pos * div)
    return pe


def build_taug(embed_table, w_score):
    table = np.asarray(embed_table, dtype=np.float32)
    w = np.asarray(w_score, dtype=np.float32).reshape(EMBED)
    taug = np.zeros((128, 2, NELEM), dtype=np.float32)
    for h in range(2):
        rows = table[128 * h:128 * (h + 1)]
        taug[:, h, :EMBED] = rows
        taug[:, h, EMBED] = rows @ w
    return taug.astype(BF)


def build_pe_consts(w_score):
    w = np.asarray(w_score, dtype=np.float32).reshape(EMBED)
    pe = _sinusoidal_pe(MAX_BLOCK, EMBED)
    peadd = np.zeros((128, NELEM), dtype=np.float32)
    p = np.arange(128)
    peadd[:, :EMBED] = pe[p % 4]
    peadd[:, EMBED] = pe[p % 4] @ w
    pe4 = np.zeros((4, NELEM), dtype=np.float32)
    pe4[:, :EMBED] = pe
    pe4[:, EMBED] = pe @ w
    ph4t = (p[None, :] % 4 == np.arange(4)[:, None]).astype(np.float32)
    return peadd, pe4.astype(BF), ph4t.astype(BF)


def build_iotasc():
    p = np.arange(128, dtype=np.float32)
    return np.stack([p, p + 128.0], axis=1)  # [128, 2] f32


def phi_of_taul(tau_l):
    return (2 * tau_l) % 3


def build_smats():
    k = np.arange(128)
    mats = np.zeros((12, 128, 128), dtype=np.float32)
    mats[0] = 0.5 * np.eye(128, dtype=np.float32)
    mats[1] = 0.25 * (k[:, None] // 2 == k[None, :] // 2)
    mats[2] = 0.125 * (k[:, None] // 4 == k[None, :] // 4)
    for phi in range(3):
        mats[3 + phi] = (1 / 6) * ((k[:, None] + phi) // 3 == (k[None, :] + phi) // 3)
        mats[6 + phi] = (1 / 6) * ((128 + k[:, None] + phi) // 3 == (k[None, :] + phi) // 3)
        mats[9 + phi] = (1 / 6) * ((k[:, None] - 128 + phi) // 3 == (k[None, :] + phi) // 3)
    return mats.astype(BF)


def build_m2rep():
    k = np.arange(128)
    j = np.arange(64)
    m2 = (j[None, :] == k[:, None] // 2).astype(np.float32)   # [128, 64]
    return np.repeat(m2[:, :, None], GSZ, axis=2).astype(BF)  # [128, 64, 12]


def build_ids_bc(input_ids):
    """Per-core int16 [128, NPOS]: every partition holds the full id
    stream (free axis = global position), feeding the one-hot compare."""
    ids = np.asarray(input_ids).astype(np.int16)
    out = []
    for core in range(NCORES):
        row = ids[core * BLOC:(core + 1) * BLOC].reshape(NPOS)
        out.append(np.tile(row[None, :], (128, 1)))
    return out


# ---------------------------------------------------------------- device prog

def emit_program(nc, nrep=1):
    taug_d = nc.dram_tensor("taug", [128, 2, NELEM], BF16, kind="ExternalInput")
    ids_d = nc.dram_tensor("idsbc", [128, NPOS], I16, kind="ExternalInput")
    iota_d = nc.dram_tensor("iotasc", [128, 2], F32, kind="ExternalInput")
    peadd_d = nc.dram_tensor("peadd", [128, NELEM], F32, kind="ExternalInput")
    pe4_d = nc.dram_tensor("pe4", [4, NELEM], BF16, kind="ExternalInput")
    ph4t_d = nc.dram_tensor("ph4t", [4, 128], BF16, kind="ExternalInput")
    smats_d = nc.dram_tensor("smats", [12, 128, 128], BF16, kind="ExternalInput")
    m2rep_d = nc.dram_tensor("m2rep", [128, 64, GSZ], BF16, kind="ExternalInput")
    out_d = nc.dram_tensor("out", [BLOC * SEQ // 2, EMBED], F32,
                           kind="ExternalOutput")

    with tile.TileContext(nc) as tc:
        with (
            tc.tile_pool(name="consts", bufs=1) as consts,
            tc.tile_pool(name="big", bufs=1) as big,
            tc.tile_pool(name="oh", bufs=4) as ohp,
            tc.tile_pool(name="sm", bufs=2) as sm,
            tc.tile_pool(name="outsb", bufs=2) as outsb_pool,
            tc.tile_pool(name="xps", bufs=2, space="PSUM") as xps_pool,
            tc.tile_pool(name="scT_ps", bufs=1, space="PSUM") as scT_ps,
            tc.tile_pool(name="gall_ps", bufs=1, space="PSUM") as gall_ps,
            tc.tile_pool(name="out2_ps", bufs=2, space="PSUM") as out2_ps,
        ):
            # ---- constants to SBUF ----
            taug_sb = consts.tile([128, 2, NELEM], BF16, tag="taug")
            nc.sync.dma_start(taug_sb[:], taug_d.ap()[:, :, :])
            iota_sb = consts.tile([128, 2], F32, tag="iota")
            nc.sync.dma_start(iota_sb[:], iota_d.ap()[:, :])
            peadd_sb = consts.tile([128, NELEM], F32, tag="peadd")
            nc.sync.dma_start(peadd_sb[:], peadd_d.ap()[:, :])
            pe4_sb = consts.tile([4, NELEM], BF16, tag="pe4")
            nc.sync.dma_start(pe4_sb[:], pe4_d.ap()[:, :])
            ph4t_sb = consts.tile([4, 128], BF16, tag="ph4t")
            nc.sync.dma_start(ph4t_sb[:], ph4t_d.ap()[:, :])
            smats_sb = consts.tile([128, 12, 128], BF16, tag="smats")
            nc.sync.dma_start(
                smats_sb[:],
                bass.AP(tensor=smats_d, offset=0,
                        ap=[[128, 128], [128 * 128, 12], [1, 128]]))
            m2rep_sb = consts.tile([128, 64, GSZ], BF16, tag="m2rep")
            nc.sync.dma_start(m2rep_sb[:], m2rep_d.ap()[:, :, :])
            ids_sb = consts.tile([128, NPOS], I16, tag="ids")
            nc.sync.dma_start(ids_sb[:], ids_d.ap()[:, :])

            # ---- persistent big tensors ----
            X = big.tile([128, NCHUNK, NELEM], BF16, tag="X")
            d2 = big.tile([128, NCHUNK], BF16, tag="d2")       # slot-ordered
            c4T = big.tile([128, 4, NCHUNK], BF16, tag="c4T")  # [m, slot]
            C2 = big.tile([128, 4, 64, NCHUNK], BF16, tag="C2")  # [m, j, slot]
            Gsb = big.tile([128, NCHUNK, 128], BF16, tag="Gsb")

            def mmat(out_ap, mi, rhs_ap, start, stop):
                nc.tensor.matmul(out=out_ap, lhsT=smats_sb[:, mi, :],
                                 rhs=rhs_ap, start=start, stop=stop,
                                 skip_group_check=True)

            def rhsC2(m, j0, nj, s0, ns):
                # C2 slice as matmul rhs with free dims ordered (slot, j)
                base = C2[:]
                off = base.offset + (m * 64 + j0) * NCHUNK + s0
                return bass.AP(tensor=base.tensor, offset=off,
                               ap=[list(base.ap[0]), [1, ns], [NCHUNK, nj]])

            def emit_gather(g):
                g0 = g * GSZ
                for c in range(GSZ):
                    ch = g0 + c
                    oh0 = ohp.tile([128, 128], BF16, tag="oh0")
                    oh1 = ohp.tile([128, 128], BF16, tag="oh1")
                    xp = xps_pool.tile([128, NELEM], F32, tag="xps")
                    mode = EVICT_MODE[ch]
                    nc.vector.tensor_scalar(
                        out=oh0[:], in0=ids_sb[:, ch * 128:(ch + 1) * 128],
                        scalar1=iota_sb[:, 0:1], scalar2=None,
                        op0=mybir.AluOpType.is_equal)
                    nc.tensor.matmul(out=xp[:], lhsT=oh0[:],
                                     rhs=taug_sb[:, 0, :], start=True,
                                     stop=False, skip_group_check=True)
                    nc.vector.tensor_scalar(
                        out=oh1[:], in0=ids_sb[:, ch * 128:(ch + 1) * 128],
                        scalar1=iota_sb[:, 1:2], scalar2=None,
                        op0=mybir.AluOpType.is_equal)
                    nc.tensor.matmul(out=xp[:], lhsT=oh1[:],
                                     rhs=taug_sb[:, 1, :], start=False,
                                     stop=(mode == 0), skip_group_check=True)
                    if mode == 0:
                        nc.vector.tensor_tensor(out=X[:, ch, :], in0=xp[:],
                                                in1=peadd_sb[:],
                                                op=mybir.AluOpType.add)
                    else:
                        nc.tensor.matmul(out=xp[:], lhsT=ph4t_sb[:],
                                         rhs=pe4_sb[:], start=False,
                                         stop=True, skip_group_check=True)
                        if mode == 1:
                            nc.scalar.copy(X[:, ch, :], xp[:])
                        else:
                            nc.gpsimd.tensor_copy(out=X[:, ch, :], in_=xp[:])
                # d2 = 2*score, written in slot order: class c block <-
                # tau_l stride-3 run
                for c in range(3):
                    t0 = CLASS_TAUL[c][0]
                    nc.gpsimd.tensor_scalar_mul(
                        d2[:, g0 + 4 * c:g0 + 4 * c + 4].unsqueeze(2),
                        X[:, g0 + t0:g0 + t0 + 10:3, EMBED:EMBED + 1], 2.0)

            def emit_scores(g):
                g0 = g * GSZ
                # m-major psum layout [128, 4 m, 12 slot]
                scT = scT_ps.tile([128, 4, GSZ], F32, tag="scT")
                mmat(scT[:, 0, :], 0, d2[:, g0:g0 + GSZ], True, False)
                mmat(scT[:, 1, :], 1, d2[:, g0:g0 + GSZ], False, False)
                mmat(scT[:, 3, :], 2, d2[:, g0:g0 + GSZ], False, False)
                for c in range(3):
                    phi = phi_of_taul(CLASS_TAUL[c][0])
                    mmat(scT[:, 2, 4 * c:4 * c + 4], 3 + phi,
                         d2[:, g0 + 4 * c:g0 + 4 * c + 4], False, False)
                up_sc = [(0, 0, 4, 8), (1, 4, 3, 1), (2, 8, 4, 4)]
                dn_sc = [(0, 1, 3, 4), (1, 4, 4, 8), (2, 8, 4, 0)]
                for plan, base in ((up_sc, 6), (dn_sc, 9)):
                    for c, o0, on, s0 in plan:
                        phi = phi_of_taul(CLASS_TAUL[c][0])
                        mmat(scT[:, 2, o0:o0 + on], base + phi,
                             d2[:, g0 + s0:g0 + s0 + on], False, False)
                if g % 2 == 0:   # up-fix: slot 7 (tau_l 11) <- next grp slot 0
                    mmat(scT[:, 2, 7:8], 6 + phi_of_taul(11),
                         d2[:, (g + 1) * GSZ:(g + 1) * GSZ + 1], False, False)
                else:            # dn-fix: slot 0 <- prev group slot 7
                    mmat(scT[:, 2, 0:1], 9 + phi_of_taul(0),
                         d2[:, g0 - GSZ + 7:g0 - GSZ + 8], False, True)

                # softmax + calibration (scS transposed to [128, slot, m])
                scS = sm.tile([128, GSZ, 4], F32, tag="scS")
                base_ap = scT[:]
                scT_t = bass.AP(tensor=base_ap.tensor, offset=base_ap.offset,
                                ap=[list(base_ap.ap[0]), list(base_ap.ap[2]),
                                    list(base_ap.ap[1])])
                nc.scalar.copy(scS[:], scT_t)
                ex = sm.tile([128, GSZ, 4], F32, tag="ex")
                nc.scalar.activation(out=ex[:], in_=scS[:],
                                     func=mybir.ActivationFunctionType.Exp)
                Z = sm.tile([128, GSZ], F32, tag="Z")
                nc.vector.tensor_reduce(out=Z[:], in_=ex[:],
                                        axis=mybir.AxisListType.X,
                                        op=mybir.AluOpType.add)
                rz = sm.tile([128, GSZ], F32, tag="rz")
                nc.vector.reciprocal(out=rz[:], in_=Z[:])
                r = sm.tile([128, GSZ, 4], F32, tag="r")
                nc.gpsimd.tensor_tensor(
                    out=r[:], in0=ex[:],
                    in1=rz[:].unsqueeze(2).to_broadcast([128, GSZ, 4]),
                    op=mybir.AluOpType.mult)
                P = sm.tile([128, GSZ, 4, 4], F32, tag="P")
                nc.gpsimd.tensor_tensor(
                    out=P[:],
                    in0=r[:].unsqueeze(3).to_broadcast([128, GSZ, 4, 4]),
                    in1=r[:].unsqueeze(2).to_broadcast([128, GSZ, 4, 4]),
                    op=mybir.AluOpType.mult)
                E = sm.tile([128, GSZ, 4, 4], F32, tag="E")
                nc.scalar.activation(out=E[:], in_=P[:],
                                     func=mybir.ActivationFunctionType.Exp)
                D = sm.tile([128, GSZ, 4], F32, tag="D")
                nc.vector.tensor_reduce(out=D[:], in_=E[:],
                                        axis=mybir.AxisListType.X,
                                        op=mybir.AluOpType.add)
                EN = sm.tile([128, GSZ, 4, 4], F32, tag="EN")
                nc.gpsimd.tensor_tensor(
                    out=EN[:], in0=E[:],
                    in1=r[:].unsqueeze(2).to_broadcast([128, GSZ, 4, 4]),
                    op=mybir.AluOpType.mult)
                Nn = sm.tile([128, GSZ, 4], F32, tag="Nn")
                nc.vector.tensor_reduce(out=Nn[:], in_=EN[:],
                                        axis=mybir.AxisListType.X,
                                        op=mybir.AluOpType.add)
                rD = sm.tile([128, GSZ, 4], F32, tag="rD")
                nc.vector.reciprocal(out=rD[:], in_=D[:])
                # c4 written transposed into c4T[:, m, slot]
                cb = c4T[:]
                c4v = bass.AP(tensor=cb.tensor,
                              offset=cb.offset + g0,
                              ap=[list(cb.ap[0]), [1, GSZ], [NCHUNK, 4]])
                nc.gpsimd.tensor_tensor(out=c4v, in0=Nn[:], in1=rD[:],
                                        op=mybir.AluOpType.mult)

                # C2 build: per-m [j, slot] = c4T bcast * m2rep, spread
                # over DVE/Pool so the m=0 slice (first build MM input)
                # lands fast and the chain isn't serialized on one engine.
                c2b = C2[:]
                for m in range(4):
                    cs = bass.AP(tensor=cb.tensor,
                                 offset=cb.offset + m * NCHUNK + g0,
                                 ap=[list(cb.ap[0]), [0, 64], [1, GSZ]])
                    c2v = bass.AP(tensor=c2b.tensor,
                                  offset=c2b.offset + m * 64 * NCHUNK + g0,
                                  ap=[list(c2b.ap[0]), [NCHUNK, 64],
                                      [1, GSZ]])
                    eng = nc.vector if m == 0 else nc.gpsimd
                    eng.tensor_tensor(out=c2v, in0=cs, in1=m2rep_sb[:],
                                      op=mybir.AluOpType.mult)

            def emit_builds(g):
                g0 = g * GSZ
                # one merged PSUM tile, region-major so every matmul output
                # is contiguous: [mid 12x64 | low 12x32 | high 12x32].
                # 6KB = 3 banks; start=True only on the first MM touching
                # each bank in PE order (zero-region reset is bank-wide):
                # bank0 <- mid q0, bank1 <- dn q0 (low slots 1:4),
                # bank2 <- up q0 (high slots 0:4).
                gall = gall_ps.tile([128, 1536], F32, tag="gall")
                gb = gall[:]

                def gv(off, o0, on, w):
                    return bass.AP(tensor=gb.tensor,
                                   offset=gb.offset + off + o0 * w,
                                   ap=[list(gb.ap[0]), [1, on * w]])

                up_plan = {0: (0, 4, 8), 1: (4, 3, 1), 2: (8, 4, 4)}
                dn_plan = {0: (1, 3, 4), 1: (4, 4, 8), 2: (8, 4, 0)}
                for q in range(3):
                    s0 = 4 * q
                    phi = phi_of_taul(CLASS_TAUL[q][0])
                    # mid band: m=0 diag + m=1 (bs2) + m=3 (bs4) + m=2 diag
                    mid = gv(0, s0, 4, 64)
                    mmat(mid, 0, rhsC2(0, 0, 64, g0 + s0, 4), q == 0, False)
                    mmat(mid, 1, rhsC2(1, 0, 64, g0 + s0, 4), False, False)
                    mmat(mid, 2, rhsC2(3, 0, 64, g0 + s0, 4), False, False)
                    mmat(mid, 3 + phi, rhsC2(2, 0, 64, g0 + s0, 4),
                         False, True)
                    o0, on, ss = dn_plan[q]
                    mmat(gv(768, o0, on, 32), 9 + phi,
                         rhsC2(2, 32, 32, g0 + ss, on), q == 0, True)
                    o0, on, ss = up_plan[q]
                    mmat(gv(1152, o0, on, 32), 6 + phi,
                         rhsC2(2, 0, 32, g0 + ss, on), q == 0, True)
                if g % 2 == 1:
                    mmat(gv(768, 0, 1, 32), 9 + phi_of_taul(0),
                         rhsC2(2, 32, 32, g0 - GSZ + 7, 1), False, True)
                    nc.vector.memset(gv(1152, 7, 1, 32), 0.0)
                else:
                    mmat(gv(1152, 7, 1, 32), 6 + phi_of_taul(11),
                         rhsC2(2, 0, 32, g0 + GSZ, 1), False, True)
                    nc.vector.memset(gv(768, 0, 1, 32), 0.0)
                # evict the three regions to Gsb (bf16)
                midv = bass.AP(tensor=gb.tensor, offset=gb.offset,
                               ap=[list(gb.ap[0]), [64, GSZ], [1, 64]])
                lowv = bass.AP(tensor=gb.tensor, offset=gb.offset + 768,
                               ap=[list(gb.ap[0]), [32, GSZ], [1, 32]])
                highv = bass.AP(tensor=gb.tensor, offset=gb.offset + 1152,
                                ap=[list(gb.ap[0]), [32, GSZ], [1, 32]])
                nc.scalar.copy(Gsb[:, g0:g0 + GSZ, 32:96], midv)
                nc.vector.tensor_copy(out=Gsb[:, g0:g0 + GSZ, 0:32],
                                      in_=lowv)
                nc.scalar.copy(Gsb[:, g0:g0 + GSZ, 96:128], highv)

            def gsb_idx(row, tt):
                g = 2 * row + tt // GSZ
                return g * GSZ + SLOT_OF[tt % GSZ]

            def emit_big(row, pair_list, osb):
                for pair in pair_list:
                    out2 = out2_ps.tile([128, 2, EMBED], F32, tag="out2")
                    first = True
                    for idx, ot in enumerate(pair):
                        tt_e = 2 * ot
                        if tt_e < 24:
                            nc.tensor.matmul(
                                out=out2[:, idx, :],
                                lhsT=Gsb[:, gsb_idx(row, tt_e), :],
                                rhs=X[:, 24 * row + tt_e, 0:EMBED],
                                start=first, stop=False,
                                skip_group_check=True)
                            first = False
                        if tt_e - 1 >= 0:
                            nc.tensor.matmul(
                                out=out2[0:64, idx, :],
                                lhsT=Gsb[:, gsb_idx(row, tt_e - 1), 64:128],
                                rhs=X[:, 24 * row + tt_e - 1, 0:EMBED],
                                start=first, stop=True,
                                skip_group_check=True)
                            first = False
                        if tt_e + 1 < 24:
                            nc.tensor.matmul(
                                out=out2[64:128, idx, :],
                                lhsT=Gsb[:, gsb_idx(row, tt_e + 1), 0:64],
                                rhs=X[:, 24 * row + tt_e + 1, 0:EMBED],
                                start=False, stop=True,
                                skip_group_check=True)
                    ot0 = pair[0]
                    np_ = len(pair)
                    c1 = 32 if ot0 == 12 else 128
                    eng = OSB_ENG[ot0 // 2]
                    if eng == 0:
                        nc.scalar.copy(osb[0:c1, ot0:ot0 + np_, :],
                                       out2[0:c1, 0:np_, :])
                    else:
                        nc.vector.tensor_copy(out=osb[0:c1, ot0:ot0 + np_, :],
                                              in_=out2[0:c1, 0:np_, :])

            def emit_row_dma(row, osb):
                # out position t = 128*ot - 32 + p; three DMAs cover
                # (ot=0, p 32:128), (ot 1:12, all p), (ot=12, p 0:32)
                base = row * (SEQ // 2)
                nc.sync.dma_start(
                    bass.AP(tensor=out_d, offset=base * EMBED,
                            ap=[[EMBED, 96], [1, EMBED]]),
                    osb[32:128, 0, :])
                nc.sync.dma_start(
                    bass.AP(tensor=out_d, offset=(base + 96) * EMBED,
                            ap=[[EMBED, 128], [128 * EMBED, 11], [1, EMBED]]),
                    osb[:, 1:12, :])
                nc.sync.dma_start(
                    bass.AP(tensor=out_d, offset=(base + 1504) * EMBED,
                            ap=[[EMBED, 32], [1, EMBED]]),
                    osb[0:32, 12, :])

            # ---- staged pipeline ----
            for _rep in range(nrep):
                osbA = outsb_pool.tile([128, 13, EMBED], F32, tag="osb")
                osbB = outsb_pool.tile([128, 13, EMBED], F32, tag="osb")
                emit_gather(0)
                emit_gather(1)
                emit_scores(0)
                emit_gather(2)
                emit_scores(1)
                emit_builds(0)
                emit_big(0, [(0, 1), (2, 3), (4, 5)], osbA)
                emit_gather(3)
                emit_scores(2)
                emit_scores(3)
                emit_builds(1)
                emit_big(0, [(6, 7), (8, 9), (10, 11), (12,)], osbA)
                emit_row_dma(0, osbA)
                emit_builds(2)
                emit_builds(3)
                emit_big(1, [(0, 1), (2, 3), (4, 5)], osbB)
                emit_big(1, [(6, 7), (8, 9), (10, 11), (12,)], osbB)
                emit_row_dma(1, osbB)

    return nc


_CACHE = {}


def _get_nc(nrep=1):
    key = f"nc{nrep}"
    if key not in _CACHE:
        nc = bacc.Bacc("TRN2", target_bir_lowering=False, debug=False)
        emit_program(nc, nrep=nrep)
        nc.compile()
        _CACHE[key] = nc
    return _CACHE[key]


def prepare_in_maps(input_ids, embed_table, w_score, b_score=None):
    # b_score only shifts all 4 scores equally -> softmax-invariant; unused.
    taug = build_taug(embed_table, w_score)
    peadd, pe4, ph4t = build_pe_consts(w_score)
    iotasc = build_iotasc()
    smats = build_smats()
    m2rep = build_m2rep()
    ids_bc = build_ids_bc(input_ids)
    return [{"taug": taug, "idsbc": ids_bc[core], "iotasc": iotasc,
             "peadd": peadd, "pe4": pe4, "ph4t": ph4t,
             "smats": smats, "m2rep": m2rep} for core in range(NCORES)]


def assemble_out(results):
    outs = [results[c]["out"].reshape(BLOC, SEQ // 2, EMBED)
            for c in range(NCORES)]
    return np.concatenate(outs, axis=0)


def kernel(input_ids, embed_table, w_score, b_score):
    in_maps = prepare_in_maps(input_ids, embed_table, w_score, b_score)
    res = run_bass_kernel_spmd(_get_nc(), in_maps,
                               core_ids=list(range(NCORES)))
    return assemble_out(res.results)


# revision 5
# speedup vs baseline: 1.4505x; 1.4505x over previous
"""GBST Trainium2 kernel v2 (nn_GBST_42434276884940).

Self-contained: takes FULL inputs, shards batch over 8 NeuronCores
(2 rows/core), runs a Bass/Tile kernel per core, gathers full output.

v2 redesign vs v1: the indirect-DMA gather (measured ~205ns/row on HW,
~1.26ms total — the v1 bottleneck) is replaced by an on-device one-hot
matmul gather: oh[v,q] = (id[q]==v) built on DVE (int16 compare, 2-byte
4x mode), then X[pos,:] = oh.T @ taug accumulated over the two 128-row
vocab halves on the PE in bf16 (1 cyc/row vs fp32's 4). Positional
encoding is per-partition-constant (phase = p%4) and folded in either
via a K=4 accumulating matmul (pe4/ph4t) or a fused PSUM-evict add of
peadd, chosen per chunk to balance PE/DVE/ACT/Pool load. All heavy
matmuls (G build + banded apply) and elementwise run in bf16
(gate is 2e-2; bf16 path measures ~1e-3).
"""

import sys

import numpy as np
import ml_dtypes

if "/opt/trn_rl_repo" not in sys.path:
    sys.path.insert(0, "/opt/trn_rl_repo")

import concourse.bass as bass
import concourse.tile as tile
from concourse import bacc, mybir
from concourse.bass_utils import run_bass_kernel_spmd

F32 = mybir.dt.float32
BF16 = mybir.dt.bfloat16
I16 = mybir.dt.int16
BF = ml_dtypes.bfloat16

MAX_BLOCK = 4
EMBED = 256
VOCAB = 256
BATCH = 16
SEQ = 3072
NCORES = 8
BLOC = BATCH // NCORES           # 2
NPOS = BLOC * SEQ                # 6144
NCHUNK = NPOS // 128             # 48
NGROUP = 4
GSZ = NCHUNK // NGROUP           # 12
NELEM = 257                      # 256 embed + 1 score col

SLOTS = [0, 3, 6, 9, 2, 5, 8, 11, 1, 4, 7, 10]   # slot s -> tau_l
SLOT_OF = {t: s for s, t in enumerate(SLOTS)}
CLASS_TAUL = [[0, 3, 6, 9], [2, 5, 8, 11], [1, 4, 7, 10]]

# per-chunk evict mode: 0 = DVE tensor_tensor add of fp32 peadd,
# 1 = phase-matmul + ACT copy, 2 = phase-matmul + Pool copy
EVICT_MODE = [0 if (c % 8) in (0, 3, 6) else 1 for c in range(NCHUNK)]
# per-chunk one-hot compare engine: 0=DVE 1=Pool
CMP_ENG = [0 for c in range(NCHUNK)]  # TSP compares, DVE only
# per-ot engine for out2->osb copy: 0=ACT 1=DVE 2=Pool
OSB_ENG = [1, 0, 1, 0, 1, 0, 1]  # per ot-PAIR: 0=ACT 1=DVE


# ---------------------------------------------------------------- host consts

def _sinusoidal_pe(max_len, d):
    pos = np.arange(max_len, dtype=np.float32)[:, None]
    div = np.exp(np.arange(0, d, 2, dtype=np.float32) * (-np.log(10000.0) / d))
    pe = np.zeros((max_len, d), dtype=np.float32)
    pe[:, 0::2] = np.sin(pos * div)
    pe[:, 1::2] = np.cos(pos * div)
    return pe


def build_taug(embed_table, w_score):
    table = np.asarray(embed_table, dtype=np.float32)
    w = np.asarray(w_score, dtype=np.float32).reshape(EMBED)
    taug = np.zeros((128, 2, NELEM), dtype=np.float32)
    for h in range(2):
        rows = table[128 * h:128 * (h + 1)]
        taug[:, h, :EMBED] = rows
        taug[:, h, EMBED] = rows @ w
    return taug.astype(BF)


def build_pe_consts(w_score):
    w = np.asarray(w_score, dtype=np.float32).reshape(EMBED)
    pe = _sinusoidal_pe(MAX_BLOCK, EMBED)
    peadd = np.zeros((128, NELEM), dtype=np.float32)
    p = np.arange(128)
    peadd[:, :EMBED] = pe[p % 4]
    peadd[:, EMBED] = pe[p % 4] @ w
    pe4 = np.zeros((4, NELEM), dtype=np.float32)
    pe4[:, :EMBED] = pe
    pe4[:, EMBED] = pe @ w
    ph4t = (p[None, :] % 4 == np.arange(4)[:, None]).astype(np.float32)
    return peadd, pe4.astype(BF), ph4t.astype(BF)


def build_iotasc():
    p = np.arange(128, dtype=np.float32)
    return np.stack([p, p + 128.0], axis=1)  # [128, 2] f32


def phi_of_taul(tau_l):
    return (2 * tau_l) % 3


def build_smats():
    k = np.arange(128)
    mats = np.zeros((12, 128, 128), dtype=np.float32)
    mats[0] = 0.5 * np.eye(128, dtype=np.float32)
    mats[1] = 0.25 * (k[:, None] // 2 == k[None, :] // 2)
    mats[2] = 0.125 * (k[:, None] // 4 == k[None, :] // 4)
    for phi in range(3):
        mats[3 + phi] = (1 / 6) * ((k[:, None] + phi) // 3 == (k[None, :] + phi) // 3)
        mats[6 + phi] = (1 / 6) * ((128 + k[:, None] + phi) // 3 == (k[None, :] + phi) // 3)
        mats[9 + phi] = (1 / 6) * ((k[:, None] - 128 + phi) // 3 == (k[None, :] + phi) // 3)
    return mats.astype(BF)


def build_m2rep():
    k = np.arange(128)
    j = np.arange(64)
    m2 = (j[None, :] == k[:, None] // 2).astype(np.float32)   # [128, 64]
    return np.repeat(m2[:, :, None], GSZ, axis=2).astype(BF)  # [128, 64, 12]


def build_ids_bc(input_ids):
    """Per-core int16 [128, NPOS]: every partition holds the full id
    stream (free axis = global position), feeding the one-hot compare."""
    ids = np.asarray(input_ids).astype(np.int16)
    out = []
    for core in range(NCORES):
        row = ids[core * BLOC:(core + 1) * BLOC].reshape(NPOS)
        out.append(np.tile(row[None, :], (128, 1)))
    return out


# ---------------------------------------------------------------- device prog

def emit_program(nc, nrep=1):
    taug_d = nc.dram_tensor("taug", [128, 2, NELEM], BF16, kind="ExternalInput")
    ids_d = nc.dram_tensor("idsbc", [128, NPOS], I16, kind="ExternalInput")
    iota_d = nc.dram_tensor("iotasc", [128, 2], F32, kind="ExternalInput")
    peadd_d = nc.dram_tensor("peadd", [128, NELEM], F32, kind="ExternalInput")
    pe4_d = nc.dram_tensor("pe4", [4, NELEM], BF16, kind="ExternalInput")
    ph4t_d = nc.dram_tensor("ph4t", [4, 128], BF16, kind="ExternalInput")
    smats_d = nc.dram_tensor("smats", [12, 128, 128], BF16, kind="ExternalInput")
    m2rep_d = nc.dram_tensor("m2rep", [128, 64, GSZ], BF16, kind="ExternalInput")
    out_d = nc.dram_tensor("out", [BLOC * SEQ // 2, EMBED], F32,
                           kind="ExternalOutput")

    with tile.TileContext(nc) as tc:
        with (
            tc.tile_pool(name="consts", bufs=1) as consts,
            tc.tile_pool(name="big", bufs=1) as big,
            tc.tile_pool(name="oh", bufs=4) as ohp,
            tc.tile_pool(name="sm", bufs=2) as sm,
            tc.tile_pool(name="outsb", bufs=2) as outsb_pool,
            tc.tile_pool(name="xps", bufs=2, space="PSUM") as xps_pool,
            tc.tile_pool(name="scT_ps", bufs=1, space="PSUM") as scT_ps,
            tc.tile_pool(name="gall_ps", bufs=1, space="PSUM") as gall_ps,
            tc.tile_pool(name="out2_ps", bufs=2, space="PSUM") as out2_ps,
        ):
            # ---- constants to SBUF ----
            taug_sb = consts.tile([128, 2, NELEM], BF16, tag="taug")
            nc.sync.dma_start(taug_sb[:], taug_d.ap()[:, :, :])
            iota_sb = consts.tile([128, 2], F32, tag="iota")
            nc.sync.dma_start(iota_sb[:], iota_d.ap()[:, :])
            peadd_sb = consts.tile([128, NELEM], F32, tag="peadd")
            nc.sync.dma_start(peadd_sb[:], peadd_d.ap()[:, :])
            pe4_sb = consts.tile([4, NELEM], BF16, tag="pe4")
            nc.sync.dma_start(pe4_sb[:], pe4_d.ap()[:, :])
            ph4t_sb = consts.tile([4, 128], BF16, tag="ph4t")
            nc.sync.dma_start(ph4t_sb[:], ph4t_d.ap()[:, :])
            smats_sb = consts.tile([128, 12, 128], BF16, tag="smats")
            nc.sync.dma_start(
                smats_sb[:],
                bass.AP(tensor=smats_d, offset=0,
                        ap=[[128, 128], [128 * 128, 12], [1, 128]]))
            m2rep_sb = consts.tile([128, 64, GSZ], BF16, tag="m2rep")
            nc.sync.dma_start(m2rep_sb[:], m2rep_d.ap()[:, :, :])
            ids_sb = consts.tile([128, NPOS], I16, tag="ids")
            nc.sync.dma_start(ids_sb[:], ids_d.ap()[:, :])

            # ---- persistent big tensors ----
            X = big.tile([128, NCHUNK, NELEM], BF16, tag="X")
            d2 = big.tile([128, NCHUNK], BF16, tag="d2")       # slot-ordered
            c4T = big.tile([128, 4, NCHUNK], BF16, tag="c4T")  # [m, slot]
            C2 = big.tile([128, 4, 64, NCHUNK], BF16, tag="C2")  # [m, j, slot]
            Gsb = big.tile([128, NCHUNK, 128], BF16, tag="Gsb")

            def mmat(out_ap, mi, rhs_ap, start, stop):
                nc.tensor.matmul(out=out_ap, lhsT=smats_sb[:, mi, :],
                                 rhs=rhs_ap, start=start, stop=stop,
                                 skip_group_check=True)

            def rhsC2(m, j0, nj, s0, ns):
                # C2 slice as matmul rhs with free dims ordered (slot, j)
                base = C2[:]
                off = base.offset + (m * 64 + j0) * NCHUNK + s0
                return bass.AP(tensor=base.tensor, offset=off,
                               ap=[list(base.ap[0]), [1, ns], [NCHUNK, nj]])

            def emit_gather(g):
                g0 = g * GSZ
                pending = []

                def flush():
                    ch, xp = pending.pop(0)
                    if EVICT_MODE[ch] == 0:
                        nc.vector.tensor_tensor(out=X[:, ch, :], in0=xp[:],
                                                in1=peadd_sb[:],
                                                op=mybir.AluOpType.add)
                    else:
                        nc.scalar.copy(X[:, ch, :], xp[:])

                for c in range(GSZ):
                    ch = g0 + c
                    oh0 = ohp.tile([128, 128], BF16, tag="oh0")
                    oh1 = ohp.tile([128, 128], BF16, tag="oh1")
                    xp = xps_pool.tile([128, NELEM], F32, tag="xps")
                    mode = EVICT_MODE[ch]
                    nc.vector.tensor_scalar(
                        out=oh0[:], in0=ids_sb[:, ch * 128:(ch + 1) * 128],
                        scalar1=iota_sb[:, 0:1], scalar2=None,
                        op0=mybir.AluOpType.is_equal)
                    nc.tensor.matmul(out=xp[:], lhsT=oh0[:],
                                     rhs=taug_sb[:, 0, :], start=True,
                                     stop=False, skip_group_check=True)
                    nc.vector.tensor_scalar(
                        out=oh1[:], in0=ids_sb[:, ch * 128:(ch + 1) * 128],
                        scalar1=iota_sb[:, 1:2], scalar2=None,
                        op0=mybir.AluOpType.is_equal)
                    nc.tensor.matmul(out=xp[:], lhsT=oh1[:],
                                     rhs=taug_sb[:, 1, :], start=False,
                                     stop=(mode == 0), skip_group_check=True)
                    if mode != 0:
                        nc.tensor.matmul(out=xp[:], lhsT=ph4t_sb[:],
                                         rhs=pe4_sb[:], start=False,
                                         stop=True, skip_group_check=True)
                    pending.append((ch, xp))
                    if len(pending) > 1:
                        flush()
                flush()
                # d2 = 2*score, written in slot order: class c block <-
                # tau_l stride-3 run
                for c in range(3):
                    t0 = CLASS_TAUL[c][0]
                    nc.gpsimd.tensor_scalar_mul(
                        d2[:, g0 + 4 * c:g0 + 4 * c + 4].unsqueeze(2),
                        X[:, g0 + t0:g0 + t0 + 10:3, EMBED:EMBED + 1], 2.0)

            def emit_scores(g):
                g0 = g * GSZ
                # m-major psum layout [128, 4 m, 12 slot]
                scT = scT_ps.tile([128, 4, GSZ], F32, tag="scT")
                mmat(scT[:, 0, :], 0, d2[:, g0:g0 + GSZ], True, False)
                mmat(scT[:, 1, :], 1, d2[:, g0:g0 + GSZ], False, False)
                mmat(scT[:, 3, :], 2, d2[:, g0:g0 + GSZ], False, False)
                for c in range(3):
                    phi = phi_of_taul(CLASS_TAUL[c][0])
                    mmat(scT[:, 2, 4 * c:4 * c + 4], 3 + phi,
                         d2[:, g0 + 4 * c:g0 + 4 * c + 4], False, False)
                up_sc = [(0, 0, 4, 8), (1, 4, 3, 1), (2, 8, 4, 4)]
                dn_sc = [(0, 1, 3, 4), (1, 4, 4, 8), (2, 8, 4, 0)]
                for plan, base in ((up_sc, 6), (dn_sc, 9)):
                    for c, o0, on, s0 in plan:
                        phi = phi_of_taul(CLASS_TAUL[c][0])
                        mmat(scT[:, 2, o0:o0 + on], base + phi,
                             d2[:, g0 + s0:g0 + s0 + on], False, False)
                if g % 2 == 0:   # up-fix: slot 7 (tau_l 11) <- next grp slot 0
                    mmat(scT[:, 2, 7:8], 6 + phi_of_taul(11),
                         d2[:, (g + 1) * GSZ:(g + 1) * GSZ + 1], False, False)
                else:            # dn-fix: slot 0 <- prev group slot 7
                    mmat(scT[:, 2, 0:1], 9 + phi_of_taul(0),
                         d2[:, g0 - GSZ + 7:g0 - GSZ + 8], False, True)

                # softmax + calibration (scS transposed to [128, slot, m])
                scS = sm.tile([128, GSZ, 4], F32, tag="scS")
                base_ap = scT[:]
                scT_t = bass.AP(tensor=base_ap.tensor, offset=base_ap.offset,
                                ap=[list(base_ap.ap[0]), list(base_ap.ap[2]),
                                    list(base_ap.ap[1])])
                nc.scalar.copy(scS[:], scT_t)
                ex = sm.tile([128, GSZ, 4], F32, tag="ex")
                nc.scalar.activation(out=ex[:], in_=scS[:],
                                     func=mybir.ActivationFunctionType.Exp)
                Z = sm.tile([128, GSZ], F32, tag="Z")
                Z2 = sm.tile([128, GSZ, 2], F32, tag="Z2")
                nc.gpsimd.tensor_tensor(out=Z2[:], in0=ex[:, :, 0:2],
                                        in1=ex[:, :, 2:4],
                                        op=mybir.AluOpType.add)
                nc.gpsimd.tensor_tensor(out=Z[:].unsqueeze(2),
                                        in0=Z2[:, :, 0:1], in1=Z2[:, :, 1:2],
                                        op=mybir.AluOpType.add)
                rz = sm.tile([128, GSZ], F32, tag="rz")
                nc.vector.reciprocal(out=rz[:], in_=Z[:])
                r = sm.tile([128, GSZ, 4], F32, tag="r")
                nc.gpsimd.tensor_tensor(
                    out=r[:], in0=ex[:],
                    in1=rz[:].unsqueeze(2).to_broadcast([128, GSZ, 4]),
                    op=mybir.AluOpType.mult)
                P = sm.tile([128, GSZ, 4, 4], F32, tag="P")
                nc.gpsimd.tensor_tensor(
                    out=P[:],
                    in0=r[:].unsqueeze(3).to_broadcast([128, GSZ, 4, 4]),
                    in1=r[:].unsqueeze(2).to_broadcast([128, GSZ, 4, 4]),
                    op=mybir.AluOpType.mult)
                E = sm.tile([128, GSZ, 4, 4], F32, tag="E")
                nc.scalar.activation(out=E[:], in_=P[:],
                                     func=mybir.ActivationFunctionType.Exp)
                D = sm.tile([128, GSZ, 4], F32, tag="D")
                D2 = sm.tile([128, GSZ, 4, 2], F32, tag="D2")
                nc.gpsimd.tensor_tensor(out=D2[:], in0=E[:, :, :, 0:2],
                                        in1=E[:, :, :, 2:4],
                                        op=mybir.AluOpType.add)
                nc.gpsimd.tensor_tensor(out=D[:].unsqueeze(3),
                                        in0=D2[:, :, :, 0:1],
                                        in1=D2[:, :, :, 1:2],
                                        op=mybir.AluOpType.add)
                EN = sm.tile([128, GSZ, 4, 4], F32, tag="EN")
                nc.gpsimd.tensor_tensor(
                    out=EN[:], in0=E[:],
                    in1=r[:].unsqueeze(2).to_broadcast([128, GSZ, 4, 4]),
                    op=mybir.AluOpType.mult)
                Nn = sm.tile([128, GSZ, 4], F32, tag="Nn")
                N2 = sm.tile([128, GSZ, 4, 2], F32, tag="N2")
                nc.gpsimd.tensor_tensor(out=N2[:], in0=EN[:, :, :, 0:2],
                                        in1=EN[:, :, :, 2:4],
                                        op=mybir.AluOpType.add)
                nc.gpsimd.tensor_tensor(out=Nn[:].unsqueeze(3),
                                        in0=N2[:, :, :, 0:1],
                                        in1=N2[:, :, :, 1:2],
                                        op=mybir.AluOpType.add)
                rD = sm.tile([128, GSZ, 4], F32, tag="rD")
                nc.vector.reciprocal(out=rD[:], in_=D[:])
                # c4 written transposed into c4T[:, m, slot]
                cb = c4T[:]
                c4v = bass.AP(tensor=cb.tensor,
                              offset=cb.offset + g0,
                              ap=[list(cb.ap[0]), [1, GSZ], [NCHUNK, 4]])
                nc.gpsimd.tensor_tensor(out=c4v, in0=Nn[:], in1=rD[:],
                                        op=mybir.AluOpType.mult)

                # C2 build: per-m [j, slot] = c4T bcast * m2rep, spread
                # over DVE/Pool so the m=0 slice (first build MM input)
                # lands fast and the chain isn't serialized on one engine.
                c2b = C2[:]
                for m in range(4):
                    cs = bass.AP(tensor=cb.tensor,
                                 offset=cb.offset + m * NCHUNK + g0,
                                 ap=[list(cb.ap[0]), [0, 64], [1, GSZ]])
                    c2v = bass.AP(tensor=c2b.tensor,
                                  offset=c2b.offset + m * 64 * NCHUNK + g0,
                                  ap=[list(c2b.ap[0]), [NCHUNK, 64],
                                      [1, GSZ]])
                    eng = nc.vector if m == 0 else nc.gpsimd
                    eng.tensor_tensor(out=c2v, in0=cs, in1=m2rep_sb[:],
                                      op=mybir.AluOpType.mult)

            def emit_builds(g):
                g0 = g * GSZ
                # one merged PSUM tile, region-major so every matmul output
                # is contiguous: [mid 12x64 | low 12x32 | high 12x32].
                # 6KB = 3 banks; start=True only on the first MM touching
                # each bank in PE order (zero-region reset is bank-wide):
                # bank0 <- mid q0, bank1 <- dn q0 (low slots 1:4),
                # bank2 <- up q0 (high slots 0:4).
                gall = gall_ps.tile([128, 1536], F32, tag="gall")
                gb = gall[:]

                def gv(off, o0, on, w):
                    return bass.AP(tensor=gb.tensor,
                                   offset=gb.offset + off + o0 * w,
                                   ap=[list(gb.ap[0]), [1, on * w]])

                up_plan = {0: (0, 4, 8), 1: (4, 3, 1), 2: (8, 4, 4)}
                dn_plan = {0: (1, 3, 4), 1: (4, 4, 8), 2: (8, 4, 0)}
                # mid band m=0/1/3 split only at the bank boundary (slot 8):
                # bank0 <- first mid MM, bank1 <- second mid MM.
                for i, (s0, ns) in enumerate(((0, 8), (8, 4))):
                    mid = gv(0, s0, ns, 64)
                    mmat(mid, 0, rhsC2(0, 0, 64, g0 + s0, ns), True, False)
                    mmat(mid, 1, rhsC2(1, 0, 64, g0 + s0, ns), False, False)
                    mmat(mid, 2, rhsC2(3, 0, 64, g0 + s0, ns), False, False)
                for q in range(3):
                    s0 = 4 * q
                    phi = phi_of_taul(CLASS_TAUL[q][0])
                    mmat(gv(0, s0, 4, 64), 3 + phi,
                         rhsC2(2, 0, 64, g0 + s0, 4), False, True)
                    o0, on, ss = dn_plan[q]
                    mmat(gv(768, o0, on, 32), 9 + phi,
                         rhsC2(2, 32, 32, g0 + ss, on), False, True)
                    o0, on, ss = up_plan[q]
                    mmat(gv(1152, o0, on, 32), 6 + phi,
                         rhsC2(2, 0, 32, g0 + ss, on), q == 0, True)
                if g % 2 == 1:
                    mmat(gv(768, 0, 1, 32), 9 + phi_of_taul(0),
                         rhsC2(2, 32, 32, g0 - GSZ + 7, 1), False, True)
                    nc.vector.memset(gv(1152, 7, 1, 32), 0.0)
                else:
                    mmat(gv(1152, 7, 1, 32), 6 + phi_of_taul(11),
                         rhsC2(2, 0, 32, g0 + GSZ, 1), False, True)
                    nc.vector.memset(gv(768, 0, 1, 32), 0.0)
                # evict the three regions to Gsb (bf16)
                midv = bass.AP(tensor=gb.tensor, offset=gb.offset,
                               ap=[list(gb.ap[0]), [64, GSZ], [1, 64]])
                lowv = bass.AP(tensor=gb.tensor, offset=gb.offset + 768,
                               ap=[list(gb.ap[0]), [32, GSZ], [1, 32]])
                highv = bass.AP(tensor=gb.tensor, offset=gb.offset + 1152,
                                ap=[list(gb.ap[0]), [32, GSZ], [1, 32]])
                nc.scalar.copy(Gsb[:, g0:g0 + GSZ, 32:96], midv)
                nc.vector.tensor_copy(out=Gsb[:, g0:g0 + GSZ, 0:32],
                                      in_=lowv)
                nc.scalar.copy(Gsb[:, g0:g0 + GSZ, 96:128], highv)

            def gsb_idx(row, tt):
                g = 2 * row + tt // GSZ
                return g * GSZ + SLOT_OF[tt % GSZ]

            def emit_big(row, pair_list, osb):
                for pair in pair_list:
                    out2 = out2_ps.tile([128, 2, EMBED], F32, tag="out2")
                    first = True
                    for idx, ot in enumerate(pair):
                        tt_e = 2 * ot
                        if tt_e < 24:
                            nc.tensor.matmul(
                                out=out2[:, idx, :],
                                lhsT=Gsb[:, gsb_idx(row, tt_e), :],
                                rhs=X[:, 24 * row + tt_e, 0:EMBED],
                                start=first, stop=False,
                                skip_group_check=True)
                            first = False
                        if tt_e - 1 >= 0:
                            nc.tensor.matmul(
                                out=out2[0:64, idx, :],
                                lhsT=Gsb[:, gsb_idx(row, tt_e - 1), 64:128],
                                rhs=X[:, 24 * row + tt_e - 1, 0:EMBED],
                                start=first, stop=True,
                                skip_group_check=True)
                            first = False
                        if tt_e + 1 < 24:
                            nc.tensor.matmul(
                                out=out2[64:128, idx, :],
                                lhsT=Gsb[:, gsb_idx(row, tt_e + 1), 0:64],
                                rhs=X[:, 24 * row + tt_e + 1, 0:EMBED],
                                start=False, stop=True,
                                skip_group_check=True)
                    ot0 = pair[0]
                    c1 = 32 if ot0 == 12 else 128
                    nc.vector.tensor_copy(out=osb[0:c1, ot0, :],
                                          in_=out2[0:c1, 0, :])
                    if len(pair) > 1:
                        nc.scalar.copy(osb[0:c1, ot0 + 1, :],
                                       out2[0:c1, 1, :])

            def emit_row_dma(row, osb):
                # out position t = 128*ot - 32 + p; three DMAs cover
                # (ot=0, p 32:128), (ot 1:12, all p), (ot=12, p 0:32)
                base = row * (SEQ // 2)
                nc.sync.dma_start(
                    bass.AP(tensor=out_d, offset=base * EMBED,
                            ap=[[EMBED, 96], [1, EMBED]]),
                    osb[32:128, 0, :])
                nc.sync.dma_start(
                    bass.AP(tensor=out_d, offset=(base + 96) * EMBED,
                            ap=[[EMBED, 128], [128 * EMBED, 11], [1, EMBED]]),
                    osb[:, 1:12, :])
                nc.sync.dma_start(
                    bass.AP(tensor=out_d, offset=(base + 1504) * EMBED,
                            ap=[[EMBED, 32], [1, EMBED]]),
                    osb[0:32, 12, :])

            # ---- staged pipeline ----
            for _rep in range(nrep):
                osbA = outsb_pool.tile([128, 13, EMBED], F32, tag="osb")
                osbB = outsb_pool.tile([128, 13, EMBED], F32, tag="osb")
                emit_gather(0)
                emit_gather(1)
                emit_scores(0)
                emit_gather(2)
                emit_scores(1)
                emit_gather(3)
                emit_builds(0)
                emit_big(0, [(0, 1), (2, 3), (4, 5)], osbA)
                emit_scores(2)
                emit_scores(3)
                emit_builds(1)
                emit_big(0, [(6, 7), (8, 9), (10, 11), (12,)], osbA)
                emit_row_dma(0, osbA)
                emit_builds(2)
                emit_builds(3)
                emit_big(1, [(0, 1), (2, 3), (4, 5)], osbB)
                emit_big(1, [(6, 7), (8, 9), (10, 11), (12,)], osbB)
                emit_row_dma(1, osbB)

    return nc


_CACHE = {}


def _get_nc(nrep=1):
    key = f"nc{nrep}"
    if key not in _CACHE:
        nc = bacc.Bacc("TRN2", target_bir_lowering=False, debug=False)
        emit_program(nc, nrep=nrep)
        nc.compile()
        _CACHE[key] = nc
    return _CACHE[key]


def prepare_in_maps(input_ids, embed_table, w_score, b_score=None):
    # b_score only shifts all 4 scores equally -> softmax-invariant; unused.
    taug = build_taug(embed_table, w_score)
    peadd, pe4, ph4t = build_pe_consts(w_score)
    iotasc = build_iotasc()
    smats = build_smats()
    m2rep = build_m2rep()
    ids_bc = build_ids_bc(input_ids)
    return [{"taug": taug, "idsbc": ids_bc[core], "iotasc": iotasc,
             "peadd": peadd, "pe4": pe4, "ph4t": ph4t,
             "smats": smats, "m2rep": m2rep} for core in range(NCORES)]


def assemble_out(results):
    outs = [results[c]["out"].reshape(BLOC, SEQ // 2, EMBED)
            for c in range(NCORES)]
    return np.concatenate(outs, axis=0)


def kernel(input_ids, embed_table, w_score, b_score):
    in_maps = prepare_in_maps(input_ids, embed_table, w_score, b_score)
    res = run_bass_kernel_spmd(_get_nc(), in_maps,
                               core_ids=list(range(NCORES)))
    return assemble_out(res.results)
